# revision 1
# baseline (speedup 1.0000x reference)
"""MultiHeadAttention (dense, B=4 S=2048 D=1024 H=16) + residual + LayerNorm
on 8 Trainium2 NeuronCores.

Sharding: core c handles batch b=c//2 and head group g=c%2 (8 of 16 heads),
all 2048 query tokens. The output projection is row-parallel over d_in; a
pair-local ReduceScatter (cores 2b, 2b+1) sums the two half-head partial fc
outputs and scatters token halves, after which each core does bias+residual+
LayerNorm for its 1024 tokens.

Device layouts (per core):
  QT/KT: [128 part, 4 chunk, 2048 tok] bf16  = projected Q^T / K^T (d_out on
         partitions; head h lives at chunk h//2, partitions 64*(h%2)..+64)
  VO:    [128 part, 16 kchunk, 8 head, 65] bf16 = V rows with a ones column
         appended (col 64) so att@V also yields softmax denominators
  scores are computed transposed ([k, q]) so exp output feeds att@V directly;
  att@V runs V-stationary producing att_out^T [65, q] whose row 64 is the
  softmax sum; normalization multiplies by a broadcast reciprocal.
"""

import numpy as np
import ml_dtypes

import concourse.bass as bass
import concourse.mybir as mybir
import concourse.tile as tile
from concourse import bacc
from concourse.bass_utils import run_bass_kernel_spmd

BF16 = mybir.dt.bfloat16
F32 = mybir.dt.float32
AF = mybir.ActivationFunctionType
OP = mybir.AluOpType

B = 4
S = 2048  # sequence length
D = 1024  # d_model
HL = 8  # heads per core
DK = 64  # head dim
DH = HL * DK  # 512 local projection width
P = 128
KC = S // P  # 16 key chunks
QC = 2  # q halves of 1024
LN_EPS = 1e-5
SCALE = 1.0 / 8.0  # 1/sqrt(DK)

_NC_CACHE = None
_LAST_RES = None


def build_nc(dbg=False):
    nc = bacc.Bacc(
        None, target_bir_lowering=False, num_devices=8, dynamic_dma_scratch_size=2048
    )

    xqT = nc.declare_dram_parameter("xqT", [D, S], BF16, isOutput=False)
    xkT = nc.declare_dram_parameter("xkT", [D, S], BF16, isOutput=False)
    xvT = nc.declare_dram_parameter("xvT", [D, S], BF16, isOutput=False)
    wqT = nc.declare_dram_parameter("wqT", [D, DH], BF16, isOutput=False)
    wkT = nc.declare_dram_parameter("wkT", [D, DH], BF16, isOutput=False)
    wvT = nc.declare_dram_parameter("wvT", [D, DH], BF16, isOutput=False)
    woT = nc.declare_dram_parameter("woT", [DH, D], BF16, isOutput=False)
    bq_d = nc.declare_dram_parameter("bq", [DH], F32, isOutput=False)
    bk_d = nc.declare_dram_parameter("bk", [DH], F32, isOutput=False)
    bv_d = nc.declare_dram_parameter("bv", [1, DH], F32, isOutput=False)
    bo_d = nc.declare_dram_parameter("bo", [1, D], F32, isOutput=False)
    gam_d = nc.declare_dram_parameter("gamma", [1, D], F32, isOutput=False)
    bet_d = nc.declare_dram_parameter("beta", [1, D], F32, isOutput=False)
    qres_d = nc.declare_dram_parameter("q_res", [S // 2, D], F32, isOutput=False)
    out_d = nc.declare_dram_parameter("out", [S // 2, D], F32, isOutput=True)
    if dbg:
        dbg_qt = nc.declare_dram_parameter("dbg_qt", [P, 4, S], BF16, isOutput=True)
        dbg_kt = nc.declare_dram_parameter("dbg_kt", [P, 4, S], BF16, isOutput=True)
        dbg_vo = nc.declare_dram_parameter("dbg_vo", [P, KC, HL, DK + 1], BF16, isOutput=True)
        dbg_att = nc.declare_dram_parameter("dbg_att", [P, 4, S], BF16, isOutput=True)
        dbg_exp = nc.declare_dram_parameter("dbg_exp", [P, KC, 1024], BF16, isOutput=True)
        dbg_fc = nc.declare_dram_parameter("dbg_fc", [S // 2, D], F32, isOutput=True)

    # collective bounce buffers (one pair per q-half) + reciprocal broadcast bounce
    cc_in = [nc.dram_tensor(f"cc_in{i}", [S // 2, D], F32) for i in range(QC)]
    cc_out = [nc.dram_tensor(f"cc_out{i}", [S // 4, D], F32) for i in range(QC)]
    sums_dram = nc.dram_tensor("sums_dram", [HL * QC, 1024], F32)
    rec_dram = nc.dram_tensor("rec_dram", [HL * QC, 1024], F32)

    groups = [[0, 1], [2, 3], [4, 5], [6, 7]]

    with tile.TileContext(nc) as tc:
        with (
            tc.tile_pool(name="pers", bufs=1) as pers,
            tc.tile_pool(name="ps", bufs=2, space="PSUM") as ps,
        ):
            QT = pers.tile([P, 4, S], BF16, tag="QT")
            KT = pers.tile([P, 4, S], BF16, tag="KT")
            VO = pers.tile([P, KC, HL, DK + 1], BF16, tag="VO")
            ATT = pers.tile([P, 4, S], BF16, tag="ATT")
            WO = pers.tile([P, 4, D], BF16, tag="WO")
            BQK = pers.tile([P, 8], F32, tag="BQK")  # cols 0-3 bq, 4-7 bk

            nc.sync.dma_start(out=WO, in_=woT.ap().rearrange("(c p) d -> p c d", p=P))
            nc.sync.dma_start(
                out=BQK[:, 0:4], in_=bq_d.ap().rearrange("(c p) -> p c", p=P)
            )
            nc.sync.dma_start(
                out=BQK[:, 4:8], in_=bk_d.ap().rearrange("(c p) -> p c", p=P)
            )
            nc.gpsimd.memset(VO[:, :, :, DK : DK + 1], 1.0)

            # ---------------- projections ----------------
            with tc.tile_pool(name="inp", bufs=1) as inp:
                XQ = inp.tile([P, 8, S], BF16, tag="XQ")
                XK = inp.tile([P, 8, S], BF16, tag="XK")
                XV = inp.tile([P, 8, S], BF16, tag="XV")
                WQ = inp.tile([P, 8, DH], BF16, tag="WQ")
                WK = inp.tile([P, 8, DH], BF16, tag="WK")
                WV = inp.tile([P, 8, DH], BF16, tag="WV")
                BVB = inp.tile([P, DH], F32, tag="BVB")

                nc.sync.dma_start(
                    out=XV, in_=xvT.ap().rearrange("(c p) s -> p c s", p=P)
                )
                nc.sync.dma_start(
                    out=WV, in_=wvT.ap().rearrange("(c p) n -> p c n", p=P)
                )
                nc.sync.dma_start(
                    out=XQ, in_=xqT.ap().rearrange("(c p) s -> p c s", p=P)
                )
                nc.sync.dma_start(
                    out=WQ, in_=wqT.ap().rearrange("(c p) n -> p c n", p=P)
                )
                nc.sync.dma_start(
                    out=XK, in_=xkT.ap().rearrange("(c p) s -> p c s", p=P)
                )
                nc.sync.dma_start(
                    out=WK, in_=wkT.ap().rearrange("(c p) n -> p c n", p=P)
                )
                nc.sync.dma_start(out=BVB, in_=bv_d.ap().to_broadcast([P, DH]))

                # V = v @ Wv.T + bv, natural layout, sliced per head into VO
                for tokc in range(KC):
                    psv = ps.tile([P, DH], F32, tag="sc")
                    for kc in range(8):
                        nc.tensor.matmul(
                            psv,
                            lhsT=XV[:, kc, tokc * P : (tokc + 1) * P],
                            rhs=WV[:, kc, :],
                            start=(kc == 0),
                            stop=(kc == 7),
                        )
                    nc.vector.tensor_tensor(
                        VO[:, tokc, :, 0:DK],
                        psv.rearrange("p (h d) -> p h d", h=HL),
                        BVB.rearrange("p (h d) -> p h d", h=HL),
                        OP.add,
                    )

                # Q^T / K^T = W @ x^T + b (d_out on partitions)
                for which, WX, XX, outT, bcol in (
                    (0, WQ, XQ, QT, 0),
                    (1, WK, XK, KT, 4),
                ):
                    for mc in range(4):
                        for nt in range(2):
                            psq = ps.tile([P, 1024], F32, tag="sc")
                            for kc in range(8):
                                for half in range(2):
                                    nc.tensor.matmul(
                                        psq[:, half * 512 : (half + 1) * 512],
                                        lhsT=WX[:, kc, mc * P : (mc + 1) * P],
                                        rhs=XX[
                                            :,
                                            kc,
                                            nt * 1024
                                            + half * 512 : nt * 1024
                                            + (half + 1) * 512,
                                        ],
                                        start=(kc == 0),
                                        stop=(kc == 7),
                                    )
                            nc.vector.tensor_scalar_add(
                                out=outT[:, mc, nt * 1024 : (nt + 1) * 1024],
                                in0=psq,
                                scalar1=BQK[:, bcol + mc : bcol + mc + 1],
                            )

            # ---------------- attention + fc + reduce-scatter ----------------
            with (
                tc.tile_pool(name="attp", bufs=1) as attp,
                tc.tile_pool(name="late", bufs=1) as late,
            ):
                GAM = late.tile([P, D], F32, tag="GAM")
                BET = late.tile([P, D], F32, tag="BET")
                BO = late.tile([P, D], F32, tag="BO")
                nc.sync.dma_start(out=GAM, in_=gam_d.ap().to_broadcast([P, D]))
                nc.sync.dma_start(out=BET, in_=bet_d.ap().to_broadcast([P, D]))
                nc.sync.dma_start(out=BO, in_=bo_d.ap().to_broadcast([P, D]))

                for qc in range(QC):
                    qlo = qc * 1024
                    for hp in range(4):
                        expA = attp.tile([P, KC, 1024], BF16, tag="exp", bufs=2)
                        expB = attp.tile([P, KC, 1024], BF16, tag="exp", bufs=2)
                        pvA = ps.tile([DK + 1, 1024], F32, tag="pv")
                        pvB = ps.tile([DK + 1, 1024], F32, tag="pv")
                        for kc in range(KC):
                            klo = kc * P
                            psc = [None, None]
                            for hb in range(2):  # head A (2hp) / head B (2hp+1)
                                plo = hb * 64
                                sc = ps.tile([P, 1024], F32, tag="sc")
                                psc[hb] = sc
                                for half in range(2):
                                    nc.tensor.matmul(
                                        sc[:, half * 512 : (half + 1) * 512],
                                        lhsT=KT[plo : plo + 64, hp, klo : klo + P],
                                        rhs=QT[
                                            plo : plo + 64,
                                            hp,
                                            qlo + half * 512 : qlo + (half + 1) * 512,
                                        ],
                                    )
                            for hb, expT in ((0, expA), (1, expB)):
                                nc.scalar.activation(
                                    out=expT[:, kc, :],
                                    in_=psc[hb],
                                    func=AF.Exp,
                                    scale=SCALE,
                                )
                            for hb, expT, pv in ((0, expA, pvA), (1, expB, pvB)):
                                for half in range(2):
                                    nc.tensor.matmul(
                                        pv[:, half * 512 : (half + 1) * 512],
                                        lhsT=VO[:, kc, 2 * hp + hb, :],
                                        rhs=expT[
                                            :, kc, half * 512 : (half + 1) * 512
                                        ],
                                        start=(kc == 0),
                                        stop=(kc == KC - 1),
                                    )
                        if dbg and qc == 0 and hp == 0:
                            nc.sync.dma_start(out=dbg_exp[:, :, :], in_=expA)
                        # stash unnormalized att_T and the sums row
                        for hb, pv in ((0, pvA), (1, pvB)):
                            h = 2 * hp + hb
                            ridx = qc * HL + h
                            rs = attp.tile([DK + 1, 1024], F32, tag="rs")
                            nc.vector.tensor_copy(
                                rs[DK : DK + 1, :], pv[DK : DK + 1, :]
                            )
                            nc.sync.dma_start(
                                out=sums_dram[ridx : ridx + 1, :],
                                in_=rs[DK : DK + 1, :],
                            )
                            if hb == 0:
                                nc.vector.tensor_copy(
                                    ATT[0:DK, hp, qlo : qlo + 1024], pv[0:DK, :]
                                )
                            else:
                                tmpB = attp.tile([DK, 1024], BF16, tag="tmpB")
                                nc.vector.tensor_copy(tmpB, pv[0:DK, :])
                                nc.sync.dma_start(
                                    out=ATT[DK:P, hp, qlo : qlo + 1024], in_=tmpB
                                )

                    # batched reciprocal of all 8 heads' sums, then normalize
                    SU = attp.tile([HL, 1024], F32, tag="SU")
                    nc.sync.dma_start(
                        out=SU, in_=sums_dram[qc * HL : (qc + 1) * HL, :]
                    )
                    nc.vector.reciprocal(SU, SU)
                    nc.sync.dma_start(
                        out=rec_dram[qc * HL : (qc + 1) * HL, :], in_=SU
                    )
                    for hp in range(4):
                        rb128 = attp.tile([P, 1024], F32, tag="rb")
                        for hb in range(2):
                            ridx = qc * HL + 2 * hp + hb
                            nc.sync.dma_start(
                                out=rb128[hb * DK : (hb + 1) * DK, :],
                                in_=rec_dram[ridx : ridx + 1, :].to_broadcast(
                                    [DK, 1024]
                                ),
                            )
                        nc.vector.tensor_tensor(
                            ATT[:, hp, qlo : qlo + 1024],
                            ATT[:, hp, qlo : qlo + 1024],
                            rb128,
                            OP.mult,
                        )

                    if dbg and qc == 0:
                        nc.sync.dma_start(out=dbg_qt[:, :, :], in_=QT)
                        nc.sync.dma_start(out=dbg_kt[:, :, :], in_=KT)
                        nc.sync.dma_start(out=dbg_vo[:, :, :, :], in_=VO)
                        nc.sync.dma_start(out=dbg_att[:, :, :], in_=ATT)
                    # fc partial for this q half -> DRAM -> pair ReduceScatter
                    for tokc in range(8):
                        tlo = qlo + tokc * P
                        psf = ps.tile([P, D], F32, tag="sc")
                        for dinc in range(4):
                            for half in range(2):
                                nc.tensor.matmul(
                                    psf[:, half * 512 : (half + 1) * 512],
                                    lhsT=ATT[:, dinc, tlo : tlo + P],
                                    rhs=WO[:, dinc, half * 512 : (half + 1) * 512],
                                    start=(dinc == 0),
                                    stop=(dinc == 3),
                                )
                        fcs = late.tile([P, D], F32, tag="fcs", bufs=2)
                        nc.vector.tensor_copy(fcs, psf)
                        nc.sync.dma_start(
                            out=cc_in[qc][tokc * P : (tokc + 1) * P, :], in_=fcs
                        )
                    if dbg and qc == 0:
                        nc.sync.dma_start(out=dbg_fc[:, :], in_=cc_in[qc].ap())
                    nc.gpsimd.collective_compute(
                        "ReduceScatter",
                        OP.add,
                        replica_groups=groups,
                        ins=[cc_in[qc].ap().opt()],
                        outs=[cc_out[qc].ap().opt()],
                    )

                    # epilogue: +bo +residual, LayerNorm, write out
                    xts = []
                    MV = late.tile([P, 4, 2], F32, tag="MV")
                    RST = late.tile([P, 4], F32, tag="RST")
                    for tc4 in range(4):
                        xt = late.tile([P, D], F32, tag="xt", bufs=4)
                        xts.append(xt)
                        nc.sync.dma_start(
                            out=xt, in_=cc_out[qc][tc4 * P : (tc4 + 1) * P, :]
                        )
                        qr = late.tile([P, D], F32, tag="qr", bufs=2)
                        nc.sync.dma_start(
                            out=qr,
                            in_=qres_d[
                                qc * 512 + tc4 * P : qc * 512 + (tc4 + 1) * P, :
                            ],
                        )
                        nc.vector.tensor_tensor(xt, xt, BO, OP.add)
                        nc.vector.tensor_tensor(xt, xt, qr, OP.add)
                        st = late.tile([P, 2, 6], F32, tag="st", bufs=2)
                        nc.vector.bn_stats(st[:, 0, :], xt[:, 0:512])
                        nc.vector.bn_stats(st[:, 1, :], xt[:, 512:1024])
                        nc.vector.bn_aggr(MV[:, tc4, :], st)
                        nc.vector.tensor_scalar_add(
                            out=RST[:, tc4 : tc4 + 1],
                            in0=MV[:, tc4, 1:2],
                            scalar1=LN_EPS,
                        )
                    nc.vector.reciprocal(RST, RST)
                    nc.scalar.activation(out=RST, in_=RST, func=AF.Sqrt)
                    for tc4 in range(4):
                        xn = late.tile([P, D], F32, tag="xn", bufs=2)
                        nc.vector.tensor_scalar(
                            out=xn,
                            in0=xts[tc4],
                            scalar1=MV[:, tc4, 0:1],
                            scalar2=RST[:, tc4 : tc4 + 1],
                            op0=OP.subtract,
                            op1=OP.mult,
                        )
                        nc.vector.tensor_tensor(xn, xn, GAM, OP.mult)
                        nc.vector.tensor_tensor(xn, xn, BET, OP.add)
                        nc.sync.dma_start(
                            out=out_d[
                                qc * 512 + tc4 * P : qc * 512 + (tc4 + 1) * P, :
                            ],
                            in_=xn,
                        )

    nc.compile()
    return nc


def _bf16(a):
    return np.ascontiguousarray(a).astype(ml_dtypes.bfloat16)


def kernel(q, k, v, Wq, bq, Wk, bk, Wv, bv, Wo, bo, gamma, beta, _trace=False):
    global _NC_CACHE
    q = np.asarray(q, np.float32)
    k = np.asarray(k, np.float32)
    v = np.asarray(v, np.float32)
    Wq, Wk, Wv, Wo = (np.asarray(w, np.float32) for w in (Wq, Wk, Wv, Wo))
    bq, bk, bv, bo = (np.asarray(x, np.float32) for x in (bq, bk, bv, bo))
    gamma = np.asarray(gamma, np.float32)
    beta = np.asarray(beta, np.float32)

    in_maps = []
    for c in range(8):
        b, g = divmod(c, 2)
        sl = slice(g * DH, (g + 1) * DH)
        qres = np.concatenate(
            [
                q[b, g * 512 : g * 512 + 512],
                q[b, 1024 + g * 512 : 1024 + g * 512 + 512],
            ]
        )
        in_maps.append(
            {
                "xqT": _bf16(q[b].T),
                "xkT": _bf16(k[b].T),
                "xvT": _bf16(v[b].T),
                "wqT": _bf16(Wq[sl, :].T),
                "wkT": _bf16(Wk[sl, :].T),
                "wvT": _bf16(Wv[sl, :].T),
                "woT": _bf16(Wo[:, sl].T),
                "bq": bq[sl].copy(),
                "bk": bk[sl].copy(),
                "bv": bv[sl].reshape(1, DH).copy(),
                "bo": bo.reshape(1, D).copy(),
                "gamma": gamma.reshape(1, D).copy(),
                "beta": beta.reshape(1, D).copy(),
                "q_res": np.ascontiguousarray(qres),
            }
        )

    if _NC_CACHE is None:
        _NC_CACHE = build_nc()
    nc = _NC_CACHE

    kw = {}
    if _trace:
        import tempfile

        kw = dict(trace=True, tmpdir=tempfile.mkdtemp(prefix="mha_trace_"))
    res = run_bass_kernel_spmd(nc, in_maps, list(range(8)), **kw)
    global _LAST_RES
    _LAST_RES = res

    out = np.empty((B, S, D), np.float32)
    for c in range(8):
        b, g = divmod(c, 2)
        r = res.results[c]["out"]
        out[b, g * 512 : g * 512 + 512] = r[0:512]
        out[b, 1024 + g * 512 : 1024 + g * 512 + 512] = r[512:1024]

    if _trace:
        kernel._last = res  # stash for test harness
    return out

